# revision 20
# baseline (speedup 1.0000x reference)
"""AffineCoupling TRN2 kernel (v8).

Computes, for z [4_000_000, 16] fp32:
    zl = z[:, :8]; zr = z[:, 8:]
    log_s = MLP_logs(zl); b = MLP_b(zl)        (5 layers, LeakyReLU(0.01) between)
    out = concat([zl, exp(log_s) * zr + b], axis=1)

Strategy (pure data parallel over 8 NeuronCores):
 - Each core gets a 507,904-row slice (slices overlap slightly to cover 4M).
 - nat tile [128, 2048] fp32 holds 16,384 rows: nat[p, s*1024+g*16+f] =
   z[r0 + s*8192 + p*64 + g, f] (s=0..1, g=0..63); HBM I/O is
   4KB-contiguous per (partition, s).
 - Feature-major transform packs 16 rows/column: partition q = gl*8+f,
   column u*128+p, where g = gh*16+gl and u = s*2+gh.  GpSimd gathers zl
   to bf16 in one 3D-AP op; transpose-in = 8 regular bf16 matmuls against
   identity (never transpose-mode, which starves the PE clock gate).
 - The two 5-layer 8->8 MLP chains (log_s / b) run as bf16 matmuls,
   2x N=512 per layer per branch, against block-diagonal lhsT.
 - LeakyReLU: 7 instances on ACT (1-op Prelu w/ bias operand); the (4,b)
   instance on DVE as 2 ops (tensor_scalar bias+scale, then a
   scalar_tensor_tensor max — STT may read only ONE input from PSUM).
 - e = Exp(log_s + b5e) on ACT -> bf16; b-cast via DVE tensor_scalar_add;
   transpose back with 16 regular bf16 matmuls; yr = e*zr + b via DVE
   tensor_tensor ops on 3D APs, writing zr in place in nat.
 - Emission is software-pipelined 4 deep with the MLP body split across
   two iterations (L2-5 of macro B, tails of macro B-1, L0-1 of macro A,
   head of A+1 interleaved), so two MLP chains are always in flight and
   ACT/PE/DVE all have ready work ahead of any dependency stall.
"""
import os
import sys

sys.path.insert(0, "/opt/trn_rl_repo")
if "/root/.axon_site/_ro/trn_rl_repo" not in sys.path:
    sys.path.append("/root/.axon_site/_ro/trn_rl_repo")

import numpy as np

import concourse.bacc as bacc
import concourse.bass as bass
import concourse.tile as tile
from concourse import mybir
from concourse.bass import _add_dep_helper
from concourse.bass_utils import run_bass_kernel_spmd

FP = mybir.dt.float32
BF = mybir.dt.bfloat16

N_CORES = 8
BATCH = 4_000_000
ROWS_PER_MACRO = 16_384            # [128, 2048] nat tile
MACROS = 31
R = ROWS_PER_MACRO * MACROS        # 507,904 rows per core
PAD_ROWS = ROWS_PER_MACRO          # guard band: writes never touch tensor tail
NAT_BUFS = 6

STEP = 498_688
STARTS = [c * STEP for c in range(N_CORES - 1)] + [BATCH - R]

C_BIAS = 128
C_TOTAL = 138
# wmat bf16 layout: identbf (128) + 10 lhsT blocks + ones row (512) +
# 2 bias rows (128 each) for the layer-4 PSUM prefill
W_ONES = 11 * 128
W_PBIAS = W_ONES + 512
W_TOTAL = W_PBIAS + 256

LAST_RESULTS = None

ALPHA = 0.01


def _build_consts(ws_logs, bs_logs, ws_b, bs_b):
    import ml_dtypes

    ws_logs = [np.asarray(w, np.float32) for w in ws_logs]
    bs_logs = [np.asarray(b, np.float32) for b in bs_logs]
    ws_b = [np.asarray(w, np.float32) for w in ws_b]
    bs_b = [np.asarray(b, np.float32) for b in bs_b]

    consts = np.zeros((128, C_TOTAL), np.float32)
    consts[:, 0:128] = np.eye(128, dtype=np.float32)
    # bias columns: index = low 3 bits of partition q -> tile(bias, 16)
    for k in range(4):
        consts[:, C_BIAS + k] = np.tile(bs_logs[k], 16)
        consts[:, C_BIAS + 4 + k] = np.tile(bs_b[k], 16)
    consts[:, C_BIAS + 8] = np.tile(bs_logs[4], 16)   # exp bias
    consts[:, C_BIAS + 9] = np.tile(bs_b[4], 16)      # b-cast bias

    wmat = np.zeros((128, W_TOTAL), np.float32)
    wmat[:, 0:128] = np.eye(128, dtype=np.float32)
    for k in range(5):
        for bi, ws in ((0, ws_logs[k]), (1, ws_b[k])):
            blk = 1 + k * 2 + bi
            lhsT = np.zeros((128, 128), np.float32)
            for t in range(16):          # t = gl; base = t*8 in q-order
                base = t * 8
                lhsT[base:base + 8, base:base + 8] = ws.T
            wmat[:, blk * 128:(blk + 1) * 128] = lhsT
    wmat[0, W_ONES:W_ONES + 512] = 1.0
    wmat[0, W_PBIAS:W_PBIAS + 128] = np.tile(bs_logs[3], 16)
    wmat[0, W_PBIAS + 128:W_PBIAS + 256] = np.tile(bs_b[3], 16)
    wmat_bf = wmat.astype(ml_dtypes.bfloat16)
    return consts, wmat_bf


def _ap(t, offset, dims):
    """AP over tile t keeping its partition dim, explicit free dims
    [[step, count], ...] and an element offset into the free space."""
    return bass.AP(tensor=t.tensor, offset=t.offset + offset, ap=[t.ap[0]] + dims)


def _build_nc():
    nc = bacc.Bacc()
    z_d = nc.declare_dram_parameter("z", [R + PAD_ROWS, 16], FP, isOutput=False)
    c_d = nc.declare_dram_parameter("consts", [128, C_TOTAL], FP, isOutput=False)
    w_d = nc.declare_dram_parameter("wmat", [128, W_TOTAL], BF, isOutput=False)
    o_d = nc.declare_dram_parameter("out", [R + PAD_ROWS, 16], FP, isOutput=True)

    with tile.TileContext(nc) as tc:
        with (
            tc.tile_pool(name="consts", bufs=1) as cp,
            tc.tile_pool(name="nat", bufs=NAT_BUFS) as natp,
            tc.tile_pool(name="sb", bufs=2) as sbp,
            tc.tile_pool(name="pshp", bufs=3, space="PSUM") as pshp,
            tc.tile_pool(name="pstb", bufs=2, space="PSUM") as pstb,
        ):
            consts = cp.tile([128, C_TOTAL], FP)
            nc.sync.dma_start(out=consts, in_=c_d[:, :])
            wmat = cp.tile([128, W_TOTAL], BF)
            nc.sync.dma_start(out=wmat, in_=w_d[:, :])
            ident = consts[:, 0:128]
            identbf = wmat[:, 0:128]
            lhsT = {}
            for k in range(5):
                for bi, beta in ((0, "s"), (1, "b")):
                    blk = 1 + k * 2 + bi
                    lhsT[(k, beta)] = wmat[:, blk * 128:(blk + 1) * 128]
            ones_row = wmat[0:1, W_ONES:W_ONES + 512]
            pbias_row = {"s": wmat[0:1, W_PBIAS:W_PBIAS + 128],
                         "b": wmat[0:1, W_PBIAS + 128:W_PBIAS + 256]}
            bias = {}
            for k in range(4):
                bias[(k, "s")] = consts[:, C_BIAS + k:C_BIAS + k + 1]
                bias[(k, "b")] = consts[:, C_BIAS + 4 + k:C_BIAS + 5 + k]
            bias_e = consts[:, C_BIAS + 8:C_BIAS + 9]
            bias_b5 = consts[:, C_BIAS + 9:C_BIAS + 10]

            # warm up engines
            wu_ps = pstb.tile([128, 512], FP, tag="tb")
            nc.tensor.matmul(wu_ps[:, 0:128], ident, ident, start=True, stop=True)
            wu1 = sbp.tile([128, 1], FP, tag="wu", bufs=2)
            nc.scalar.copy(out=wu1, in_=bias_e)
            wu2 = sbp.tile([128, 1], FP, tag="wu")
            nc.vector.tensor_copy(out=wu2, in_=bias_e)
            wu3 = sbp.tile([128, 1], FP, tag="wu")
            nc.gpsimd.tensor_copy(out=wu3, in_=bias_e)

            nats = {}
            x0s = {}
            curs = {}
            ebs = {}
            tails = {}
            tail_dmas = []

            def dma_in(m):
                nat = natp.tile([128, 2048], FP, tag="nat")
                nats[m] = nat
                nc.sync.dma_start(
                    out=nat.rearrange("p (s g f) -> p s g f", s=2, g=64, f=16),
                    in_=z_d[m * ROWS_PER_MACRO:(m + 1) * ROWS_PER_MACRO, :]
                    .rearrange("(s p g) f -> p s g f", s=2, p=128, g=64),
                )

            def head(m):
                nat = nats[m]
                # zl gather-cast (GpSimd, one 3D-AP op):
                # natzl[p, u*128+gl*8+f] = nat[p, u*256+gl*16+f]
                natzl = sbp.tile([128, 1024], BF, tag="nzl", bufs=2)
                nc.gpsimd.tensor_copy(
                    out=_ap(natzl, 0, [[128, 8], [8, 16], [1, 8]]),
                    in_=_ap(nat, 0, [[256, 8], [16, 16], [1, 8]]),
                )
                # transpose-in -> x0[q, u*128+p], q = gl*8+f
                x0 = sbp.tile([128, 1024], BF, tag="x0", bufs=3)
                x0ps = pshp.tile([128, 1024], FP, tag="hp")
                for u in range(8):
                    nc.tensor.matmul(
                        x0ps[:, u * 128:(u + 1) * 128],
                        natzl[:, u * 128:(u + 1) * 128],
                        identbf, start=True, stop=True,
                    )
                nc.vector.tensor_copy(out=x0, in_=x0ps)
                x0s[m] = x0
                curs[m] = {"s": x0, "b": x0}

            def body_layer(m, k):
                cur = curs[m]
                hps = {}
                for beta in ("s", "b"):
                    hp = pshp.tile([128, 1024], FP, tag="hp")
                    for c in range(2):
                        nc.tensor.matmul(
                            hp[:, c * 512:(c + 1) * 512],
                            lhsT[(k, beta)],
                            cur[beta][:, c * 512:(c + 1) * 512],
                            start=True, stop=True,
                        )
                    hps[beta] = hp
                for beta in ("s", "b"):
                    hout = sbp.tile([128, 1024], BF, tag="h", bufs=12)
                    if (k, beta) == (3, "b"):
                        # DVE 2-op leaky: t1 = (u+bias)*0.01 ; max(u+bias, t1)
                        # (STT may read only one input from PSUM)
                        t1 = sbp.tile([128, 1024], BF, tag="t", bufs=2)
                        nc.vector.tensor_scalar(
                            out=t1, in0=hps[beta], scalar1=bias[(k, beta)],
                            scalar2=ALPHA,
                            op0=mybir.AluOpType.add, op1=mybir.AluOpType.mult,
                        )
                        nc.vector.scalar_tensor_tensor(
                            out=hout, in0=hps[beta], scalar=bias[(k, beta)],
                            in1=t1,
                            op0=mybir.AluOpType.add, op1=mybir.AluOpType.max,
                        )
                    else:
                        nc.scalar.activation(
                            out=hout, in_=hps[beta],
                            func=mybir.ActivationFunctionType.Prelu,
                            bias=bias[(k, beta)], scale=1.0, alpha=ALPHA,
                        )
                    cur[beta] = hout

            def body_l5(m):
                cur = curs.pop(m)
                x0s.pop(m)
                hp5 = {}
                for beta in ("s", "b"):
                    hp = pshp.tile([128, 1024], FP, tag="hp")
                    for c in range(2):
                        nc.tensor.matmul(
                            hp[:, c * 512:(c + 1) * 512],
                            lhsT[(4, beta)],
                            cur[beta][:, c * 512:(c + 1) * 512],
                            start=True, stop=True,
                        )
                    hp5[beta] = hp
                ebf_e = sbp.tile([128, 1024], BF, tag="eb", bufs=4)
                nc.scalar.activation(
                    out=ebf_e, in_=hp5["s"],
                    func=mybir.ActivationFunctionType.Exp,
                    bias=bias_e, scale=1.0,
                )
                ebf_b = sbp.tile([128, 1024], BF, tag="eb", bufs=4)
                nc.vector.tensor_scalar_add(
                    out=ebf_b, in0=hp5["b"], scalar1=bias_b5)
                ebs[m] = (ebf_e, ebf_b)

            def tail_half(m, h):
                nat = nats[m]
                ebf_e, ebf_b = ebs[m]
                # eT[p, j*128 + q] ; q = gl*8+o
                # nat zr col = u*256 + gl*16 + 8 + o  (u = h*4+j)
                eT = pstb.tile([128, 512], FP, tag="tb")
                bT = pstb.tile([128, 512], FP, tag="tb")
                for j in range(4):
                    u = h * 4 + j
                    nc.tensor.matmul(
                        eT[:, j * 128:(j + 1) * 128],
                        ebf_e[:, u * 128:(u + 1) * 128],
                        identbf, start=True, stop=True,
                    )
                    nc.tensor.matmul(
                        bT[:, j * 128:(j + 1) * 128],
                        ebf_b[:, u * 128:(u + 1) * 128],
                        identbf, start=True, stop=True,
                    )
                et_ap = _ap(eT, 0, [[128, 4], [8, 16], [1, 8]])
                bt_ap = _ap(bT, 0, [[128, 4], [8, 16], [1, 8]])
                zr_ap = _ap(nat, h * 1024 + 8, [[256, 4], [16, 16], [1, 8]])
                tmp = sbp.tile([128, 512], FP, tag="tmp", bufs=4)
                tmp_ap = _ap(tmp, 0, [[128, 4], [8, 16], [1, 8]])
                nc.vector.tensor_mul(out=tmp_ap, in0=et_ap, in1=zr_ap)
                nc.vector.tensor_add(out=zr_ap, in0=tmp_ap, in1=bt_ap)

            def tail_out(m):
                nat = nats.pop(m)
                ebs.pop(m)
                out_dma = nc.sync.dma_start(
                    out=o_d[m * ROWS_PER_MACRO:(m + 1) * ROWS_PER_MACRO, :]
                    .rearrange("(s p g) f -> p s g f", s=2, p=128, g=64),
                    in_=nat.rearrange("p (s g f) -> p s g f", s=2, g=64, f=16),
                )
                if m >= MACROS - NAT_BUFS:
                    tail_dmas.append(out_dma)

            # ---- software-pipelined emission, body split across two
            # iterations so two MLP chains are always in flight:
            #   iter: L2-3(B=it-2) | tails(it-3) | L0-1(A=it-1) | head(it)
            dma_in(0)
            dma_in(1)
            for it in range(MACROS + 3):
                blA = it - 1         # early body (layers 0-1)
                blB = it - 2         # late body (layers 2-3, L5, exp)
                tl = it - 3          # tail macro
                has_a = 0 <= blA < MACROS
                has_b = 0 <= blB < MACROS
                has_t = 0 <= tl < MACROS
                if has_b:
                    body_layer(blB, 2)
                if has_t:
                    tail_half(tl, 0)
                if has_a:
                    body_layer(blA, 0)
                if has_b:
                    body_layer(blB, 3)
                if has_t:
                    tail_half(tl, 1)
                if has_a:
                    body_layer(blA, 1)
                if it < MACROS:
                    head(it)
                if has_b:
                    body_l5(blB)
                if it + 2 < MACROS:
                    dma_in(it + 2)
                if has_t:
                    tail_out(tl)

            flush = sbp.tile([128, 1], FP, tag="wu")
            fl = nc.vector.tensor_copy(out=flush, in_=bias_e)
            for dma in tail_dmas:
                _add_dep_helper(fl.ins, dma.ins, sync=True,
                                reason="drain tail out-DMAs before kernel end")

    nc.finalize()
    return nc


_NC_CACHE = None


def kernel(z, ws_logs, bs_logs, ws_b, bs_b):
    global _NC_CACHE, LAST_RESULTS
    z = np.asarray(z, np.float32)
    assert z.shape == (BATCH, 16)
    consts, wmat_bf = _build_consts(ws_logs, bs_logs, ws_b, bs_b)

    if _NC_CACHE is None:
        _NC_CACHE = _build_nc()
    nc = _NC_CACHE

    in_maps = []
    for s in STARTS:
        zp = np.zeros((R + PAD_ROWS, 16), np.float32)
        zp[:R] = z[s:s + R]
        in_maps.append({"z": zp, "consts": consts, "wmat": wmat_bf})
    trace = bool(os.environ.get("AFFINE_TRACE"))
    res = run_bass_kernel_spmd(nc, in_maps, core_ids=list(range(N_CORES)), trace=trace)
    LAST_RESULTS = res

    out = np.empty((BATCH, 16), np.float32)
    for c in range(N_CORES):
        out[STARTS[c]:STARTS[c] + R] = res.results[c]["out"][:R]
    return out


# revision 24
# speedup vs baseline: 1.0013x; 1.0013x over previous
"""AffineCoupling TRN2 kernel (v8).

Computes, for z [4_000_000, 16] fp32:
    zl = z[:, :8]; zr = z[:, 8:]
    log_s = MLP_logs(zl); b = MLP_b(zl)        (5 layers, LeakyReLU(0.01) between)
    out = concat([zl, exp(log_s) * zr + b], axis=1)

Strategy (pure data parallel over 8 NeuronCores):
 - Each core gets a 507,904-row slice (slices overlap slightly to cover 4M).
 - nat tile [128, 2048] fp32 holds 16,384 rows: nat[p, s*1024+g*16+f] =
   z[r0 + s*8192 + p*64 + g, f] (s=0..1, g=0..63); HBM I/O is
   4KB-contiguous per (partition, s).
 - Feature-major transform packs 16 rows/column: partition q = gl*8+f,
   column u*128+p, where g = gh*16+gl and u = s*2+gh.  GpSimd gathers zl
   to bf16 in one 3D-AP op; transpose-in = 8 regular bf16 matmuls against
   identity (never transpose-mode, which starves the PE clock gate).
 - The two 5-layer 8->8 MLP chains (log_s / b) run as bf16 matmuls,
   2x N=512 per layer per branch, against block-diagonal lhsT.
 - LeakyReLU: 7 instances on ACT (1-op Prelu w/ bias operand); the (4,b)
   instance on DVE as 2 ops (tensor_scalar bias+scale, then a
   scalar_tensor_tensor max — STT may read only ONE input from PSUM).
 - e = Exp(log_s + b5e) on ACT -> bf16; b-cast via DVE tensor_scalar_add;
   transpose back with 16 regular bf16 matmuls; yr = e*zr + b via DVE
   tensor_tensor ops on 3D APs, writing zr in place in nat.
 - Emission is software-pipelined 4 deep with the MLP body split across
   two iterations (L2-5 of macro B, tails of macro B-1, L0-1 of macro A,
   head of A+1 interleaved), so two MLP chains are always in flight and
   ACT/PE/DVE all have ready work ahead of any dependency stall.
"""
import os
import sys

sys.path.insert(0, "/opt/trn_rl_repo")
if "/root/.axon_site/_ro/trn_rl_repo" not in sys.path:
    sys.path.append("/root/.axon_site/_ro/trn_rl_repo")

import numpy as np

import concourse.bacc as bacc
import concourse.bass as bass
import concourse.tile as tile
from concourse import mybir
from concourse.bass import _add_dep_helper
from concourse.bass_utils import run_bass_kernel_spmd

FP = mybir.dt.float32
BF = mybir.dt.bfloat16

N_CORES = 8
BATCH = 4_000_000
ROWS_PER_MACRO = 16_384            # [128, 2048] nat tile
MACROS = 31
R = ROWS_PER_MACRO * MACROS        # 507,904 rows per core
PAD_ROWS = ROWS_PER_MACRO          # guard band: writes never touch tensor tail
NAT_BUFS = 6

STEP = 498_688
STARTS = [c * STEP for c in range(N_CORES - 1)] + [BATCH - R]

C_BIAS = 128
C_TOTAL = 138
# wmat bf16 layout: identbf (128) + 10 lhsT blocks + ones row (512) +
# 2 bias rows (128 each) for the layer-4 PSUM prefill
W_ONES = 11 * 128
W_PBIAS = W_ONES + 512
W_TOTAL = W_PBIAS + 256

LAST_RESULTS = None

ALPHA = 0.01


def _build_consts(ws_logs, bs_logs, ws_b, bs_b):
    import ml_dtypes

    ws_logs = [np.asarray(w, np.float32) for w in ws_logs]
    bs_logs = [np.asarray(b, np.float32) for b in bs_logs]
    ws_b = [np.asarray(w, np.float32) for w in ws_b]
    bs_b = [np.asarray(b, np.float32) for b in bs_b]

    consts = np.zeros((128, C_TOTAL), np.float32)
    consts[:, 0:128] = np.eye(128, dtype=np.float32)
    # bias columns: index = low 3 bits of partition q -> tile(bias, 16)
    for k in range(4):
        consts[:, C_BIAS + k] = np.tile(bs_logs[k], 16)
        consts[:, C_BIAS + 4 + k] = np.tile(bs_b[k], 16)
    consts[:, C_BIAS + 8] = np.tile(bs_logs[4], 16)   # exp bias
    consts[:, C_BIAS + 9] = np.tile(bs_b[4], 16)      # b-cast bias

    wmat = np.zeros((128, W_TOTAL), np.float32)
    wmat[:, 0:128] = np.eye(128, dtype=np.float32)
    for k in range(5):
        for bi, ws in ((0, ws_logs[k]), (1, ws_b[k])):
            blk = 1 + k * 2 + bi
            lhsT = np.zeros((128, 128), np.float32)
            for t in range(16):          # t = gl; base = t*8 in q-order
                base = t * 8
                lhsT[base:base + 8, base:base + 8] = ws.T
            wmat[:, blk * 128:(blk + 1) * 128] = lhsT
    wmat[0, W_ONES:W_ONES + 512] = 1.0
    wmat[0, W_PBIAS:W_PBIAS + 128] = np.tile(bs_logs[3], 16)
    wmat[0, W_PBIAS + 128:W_PBIAS + 256] = np.tile(bs_b[3], 16)
    wmat_bf = wmat.astype(ml_dtypes.bfloat16)
    return consts, wmat_bf


def _ap(t, offset, dims):
    """AP over tile t keeping its partition dim, explicit free dims
    [[step, count], ...] and an element offset into the free space."""
    return bass.AP(tensor=t.tensor, offset=t.offset + offset, ap=[t.ap[0]] + dims)


def _build_nc():
    nc = bacc.Bacc()
    z_d = nc.declare_dram_parameter("z", [R + PAD_ROWS, 16], FP, isOutput=False)
    c_d = nc.declare_dram_parameter("consts", [128, C_TOTAL], FP, isOutput=False)
    w_d = nc.declare_dram_parameter("wmat", [128, W_TOTAL], BF, isOutput=False)
    o_d = nc.declare_dram_parameter("out", [R + PAD_ROWS, 16], FP, isOutput=True)

    with tile.TileContext(nc) as tc:
        with (
            tc.tile_pool(name="consts", bufs=1) as cp,
            tc.tile_pool(name="nat", bufs=NAT_BUFS) as natp,
            tc.tile_pool(name="sb", bufs=2) as sbp,
            tc.tile_pool(name="pshp", bufs=3, space="PSUM") as pshp,
            tc.tile_pool(name="pstb", bufs=2, space="PSUM") as pstb,
        ):
            consts = cp.tile([128, C_TOTAL], FP)
            nc.sync.dma_start(out=consts, in_=c_d[:, :])
            wmat = cp.tile([128, W_TOTAL], BF)
            nc.sync.dma_start(out=wmat, in_=w_d[:, :])
            ident = consts[:, 0:128]
            identbf = wmat[:, 0:128]
            lhsT = {}
            for k in range(5):
                for bi, beta in ((0, "s"), (1, "b")):
                    blk = 1 + k * 2 + bi
                    lhsT[(k, beta)] = wmat[:, blk * 128:(blk + 1) * 128]
            ones_row = wmat[0:1, W_ONES:W_ONES + 512]
            pbias_row = {"s": wmat[0:1, W_PBIAS:W_PBIAS + 128],
                         "b": wmat[0:1, W_PBIAS + 128:W_PBIAS + 256]}
            bias = {}
            for k in range(4):
                bias[(k, "s")] = consts[:, C_BIAS + k:C_BIAS + k + 1]
                bias[(k, "b")] = consts[:, C_BIAS + 4 + k:C_BIAS + 5 + k]
            bias_e = consts[:, C_BIAS + 8:C_BIAS + 9]
            bias_b5 = consts[:, C_BIAS + 9:C_BIAS + 10]

            # warm up engines
            wu_ps = pstb.tile([128, 512], FP, tag="tb")
            nc.tensor.matmul(wu_ps[:, 0:128], ident, ident, start=True, stop=True)
            wu1 = sbp.tile([128, 1], FP, tag="wu", bufs=2)
            nc.scalar.copy(out=wu1, in_=bias_e)
            wu2 = sbp.tile([128, 1], FP, tag="wu")
            nc.vector.tensor_copy(out=wu2, in_=bias_e)
            wu3 = sbp.tile([128, 1], FP, tag="wu")
            nc.gpsimd.tensor_copy(out=wu3, in_=bias_e)

            nats = {}
            x0s = {}
            curs = {}
            ebs = {}
            tails = {}
            tail_dmas = []

            def dma_in(m):
                nat = natp.tile([128, 2048], FP, tag="nat")
                nats[m] = nat
                nc.sync.dma_start(
                    out=nat.rearrange("p (s g f) -> p s g f", s=2, g=64, f=16),
                    in_=z_d[m * ROWS_PER_MACRO:(m + 1) * ROWS_PER_MACRO, :]
                    .rearrange("(s p g) f -> p s g f", s=2, p=128, g=64),
                )

            def head(m):
                nat = nats[m]
                # zl gather-cast (GpSimd, one 3D-AP op):
                # natzl[p, u*128+gl*8+f] = nat[p, u*256+gl*16+f]
                natzl = sbp.tile([128, 1024], BF, tag="nzl", bufs=2)
                nc.gpsimd.tensor_copy(
                    out=_ap(natzl, 0, [[128, 8], [8, 16], [1, 8]]),
                    in_=_ap(nat, 0, [[256, 8], [16, 16], [1, 8]]),
                )
                # transpose-in -> x0[q, u*128+p], q = gl*8+f
                x0 = sbp.tile([128, 1024], BF, tag="x0", bufs=3)
                x0ps = pshp.tile([128, 1024], FP, tag="hp")
                for u in range(8):
                    nc.tensor.matmul(
                        x0ps[:, u * 128:(u + 1) * 128],
                        natzl[:, u * 128:(u + 1) * 128],
                        identbf, start=True, stop=True,
                    )
                nc.vector.tensor_copy(out=x0, in_=x0ps)
                x0s[m] = x0
                curs[m] = {"s": x0, "b": x0}

            def body_layer(m, k):
                cur = curs[m]
                hps = {}
                for beta in ("s", "b"):
                    hp = pshp.tile([128, 1024], FP, tag="hp")
                    for c in range(2):
                        nc.tensor.matmul(
                            hp[:, c * 512:(c + 1) * 512],
                            lhsT[(k, beta)],
                            cur[beta][:, c * 512:(c + 1) * 512],
                            start=True, stop=True,
                        )
                    hps[beta] = hp
                for beta in ("s", "b"):
                    hout = sbp.tile([128, 1024], BF, tag="h", bufs=12)
                    if (k, beta) == (3, "b"):
                        # DVE 2-op leaky: t1 = (u+bias)*0.01 ; max(u+bias, t1)
                        # (STT may read only one input from PSUM)
                        t1 = sbp.tile([128, 1024], BF, tag="t", bufs=2)
                        nc.vector.tensor_scalar(
                            out=t1, in0=hps[beta], scalar1=bias[(k, beta)],
                            scalar2=ALPHA,
                            op0=mybir.AluOpType.add, op1=mybir.AluOpType.mult,
                        )
                        nc.vector.scalar_tensor_tensor(
                            out=hout, in0=hps[beta], scalar=bias[(k, beta)],
                            in1=t1,
                            op0=mybir.AluOpType.add, op1=mybir.AluOpType.max,
                        )
                    else:
                        nc.scalar.activation(
                            out=hout, in_=hps[beta],
                            func=mybir.ActivationFunctionType.Prelu,
                            bias=bias[(k, beta)], scale=1.0, alpha=ALPHA,
                        )
                    cur[beta] = hout

            hp5s = {}

            def body_l5_mm(m):
                cur = curs.pop(m)
                x0s.pop(m)
                hp5 = {}
                for beta in ("s", "b"):
                    hp = pshp.tile([128, 1024], FP, tag="hp")
                    for c in range(2):
                        nc.tensor.matmul(
                            hp[:, c * 512:(c + 1) * 512],
                            lhsT[(4, beta)],
                            cur[beta][:, c * 512:(c + 1) * 512],
                            start=True, stop=True,
                        )
                    hp5[beta] = hp
                hp5s[m] = hp5

            def exp_bcast(m):
                # emitted at the START of m's tail iteration: the hp5 MMs
                # are long done, so ACT/DVE have ready work at the boundary
                hp5 = hp5s.pop(m)
                ebf_e = sbp.tile([128, 1024], BF, tag="eb", bufs=4)
                nc.scalar.activation(
                    out=ebf_e, in_=hp5["s"],
                    func=mybir.ActivationFunctionType.Exp,
                    bias=bias_e, scale=1.0,
                )
                ebf_b = sbp.tile([128, 1024], BF, tag="eb", bufs=4)
                nc.vector.tensor_scalar_add(
                    out=ebf_b, in0=hp5["b"], scalar1=bias_b5)
                ebs[m] = (ebf_e, ebf_b)

            def tail_half(m, h):
                nat = nats[m]
                ebf_e, ebf_b = ebs[m]
                # eT[p, j*128 + q] ; q = gl*8+o
                # nat zr col = u*256 + gl*16 + 8 + o  (u = h*4+j)
                eT = pstb.tile([128, 512], FP, tag="tb")
                bT = pstb.tile([128, 512], FP, tag="tb")
                for j in range(4):
                    u = h * 4 + j
                    nc.tensor.matmul(
                        eT[:, j * 128:(j + 1) * 128],
                        ebf_e[:, u * 128:(u + 1) * 128],
                        identbf, start=True, stop=True,
                    )
                    nc.tensor.matmul(
                        bT[:, j * 128:(j + 1) * 128],
                        ebf_b[:, u * 128:(u + 1) * 128],
                        identbf, start=True, stop=True,
                    )
                et_ap = _ap(eT, 0, [[128, 4], [8, 16], [1, 8]])
                bt_ap = _ap(bT, 0, [[128, 4], [8, 16], [1, 8]])
                zr_ap = _ap(nat, h * 1024 + 8, [[256, 4], [16, 16], [1, 8]])
                tmp = sbp.tile([128, 512], FP, tag="tmp", bufs=4)
                tmp_ap = _ap(tmp, 0, [[128, 4], [8, 16], [1, 8]])
                nc.vector.tensor_mul(out=tmp_ap, in0=et_ap, in1=zr_ap)
                nc.vector.tensor_add(out=zr_ap, in0=tmp_ap, in1=bt_ap)

            def tail_out(m):
                nat = nats.pop(m)
                ebs.pop(m)
                out_dma = nc.sync.dma_start(
                    out=o_d[m * ROWS_PER_MACRO:(m + 1) * ROWS_PER_MACRO, :]
                    .rearrange("(s p g) f -> p s g f", s=2, p=128, g=64),
                    in_=nat.rearrange("p (s g f) -> p s g f", s=2, g=64, f=16),
                )
                if m >= MACROS - NAT_BUFS:
                    tail_dmas.append(out_dma)

            # ---- software-pipelined emission, body split across two
            # iterations so two MLP chains are always in flight:
            #   iter: L2-3(B=it-2) | tails(it-3) | L0-1(A=it-1) | head(it)
            dma_in(0)
            dma_in(1)
            for it in range(MACROS + 3):
                blA = it - 1         # early body (layers 0-1)
                blB = it - 2         # late body (layers 2-3, L5, exp)
                tl = it - 3          # tail macro
                has_a = 0 <= blA < MACROS
                has_b = 0 <= blB < MACROS
                has_t = 0 <= tl < MACROS
                if has_t:
                    exp_bcast(tl)
                if has_b:
                    body_layer(blB, 2)
                if has_t:
                    tail_half(tl, 0)
                if has_a:
                    body_layer(blA, 0)
                if has_b:
                    body_layer(blB, 3)
                if has_t:
                    tail_half(tl, 1)
                if has_a:
                    body_layer(blA, 1)
                if it < MACROS:
                    head(it)
                if has_b:
                    body_l5_mm(blB)
                if it + 2 < MACROS:
                    dma_in(it + 2)
                if has_t:
                    tail_out(tl)

            flush = sbp.tile([128, 1], FP, tag="wu")
            fl = nc.vector.tensor_copy(out=flush, in_=bias_e)
            for dma in tail_dmas:
                _add_dep_helper(fl.ins, dma.ins, sync=True,
                                reason="drain tail out-DMAs before kernel end")

    nc.finalize()
    return nc


_NC_CACHE = None


def kernel(z, ws_logs, bs_logs, ws_b, bs_b):
    global _NC_CACHE, LAST_RESULTS
    z = np.asarray(z, np.float32)
    assert z.shape == (BATCH, 16)
    consts, wmat_bf = _build_consts(ws_logs, bs_logs, ws_b, bs_b)

    if _NC_CACHE is None:
        _NC_CACHE = _build_nc()
    nc = _NC_CACHE

    in_maps = []
    for s in STARTS:
        zp = np.zeros((R + PAD_ROWS, 16), np.float32)
        zp[:R] = z[s:s + R]
        in_maps.append({"z": zp, "consts": consts, "wmat": wmat_bf})
    trace = bool(os.environ.get("AFFINE_TRACE"))
    res = run_bass_kernel_spmd(nc, in_maps, core_ids=list(range(N_CORES)), trace=trace)
    LAST_RESULTS = res

    out = np.empty((BATCH, 16), np.float32)
    for c in range(N_CORES):
        out[STARTS[c]:STARTS[c] + R] = res.results[c]["out"][:R]
    return out
